# revision 41
# baseline (speedup 1.0000x reference)
"""NT-Xent contrastive loss on 8 Trainium2 NeuronCores (Bass/Tile), v2.

Strategy (no collectives; slab-cover SPMD as v1, rebuilt for engine density):
  * Host pre-transposes embedded_data to embT [2048, 8192] and converts
    emb/W to bf16 (halves DMA; rel tol 2e-2 leaves ~100x margin).
  * Slab cover: core c loads slabs S_c = {c, c+1, c+2, c+4} (mod 8) of emb
    (16 MiB/core bf16).  Every slab pair meets on some core; each core
    computes 5 sim blocks of 1024x1024 (diag + 4 pairs).
  * Head: p_h = W.T @ embT chunkwise (bf16, FWL), +b into bf16 t_h.
  * Normalize: normsq via ones[128,128]-matmul (replicates norms across all
    128 partitions), then r = exp(-0.5*ln(normsq)) on ACT -- Ln and Exp share
    one table set, so the WHOLE kernel needs a single ACT table load.
    t_on = t_h * r (bf16).
  * Sim blocks: psum [128,1024] <- 4 bf16 matmuls; ACT exp(10x) with fused
    per-row accum (rowsums); colsums accumulated on DVE into [128,1024] f32
    tiles shipped to host (host reduces the 128 partitions).
  * Diag: sim_ii == 1 by construction, host subtracts e^10 (no mask pass).
  * pos: elementwise t_on0*t_on3 + ones-column matmul -> possim;
    log(pos) = 10*possim exactly.
  * Host (fp64) combines row/col partials; loss = -mean(10*possim - log(neg)).
"""
import numpy as np

SLOTS = [(c, (c + 1) % 8, (c + 2) % 8, (c + 4) % 8) for c in range(8)]
# blocks in local slot coords: (stationary, moving). B0 = diag.
BLOCKS = [(0, 0), (0, 1), (0, 2), (1, 3), (0, 3)]

_CACHE = {}


def _build():
    if "nc" in _CACHE:
        return _CACHE["nc"]
    import concourse.bacc as bacc
    import concourse.tile as tile
    import concourse.mybir as mybir

    import math
    F32, BF16, F8 = mybir.dt.float32, mybir.dt.bfloat16, mybir.dt.float8e4
    AF = mybir.ActivationFunctionType
    ALU = mybir.AluOpType
    DR = mybir.MatmulPerfMode.DoubleRow

    nc = bacc.Bacc("TRN2", num_devices=8, debug=False)

    # Pin Ln+Exp to the one table set containing both: strip them from every
    # other set in the (cached) table dict so the table-load inserter cannot
    # alternate between exp_and_others and natural_log (saves ~2.7us per
    # switch and keeps the normalize chain off the ACT critical path).
    # Key order is preserved, so act_func_set_id indices stay valid.
    tables = bacc.get_activation_tables(nc.m.arch)
    if "natural_log_exp_and_others" in tables:
        for name, funcs in tables.items():
            if name != "natural_log_exp_and_others":
                funcs.discard(AF.Exp)
                funcs.discard(AF.Ln)

    a_emb = nc.dram_tensor("embT", [2048, 4096], F8, kind="ExternalInput").ap()
    a_W = nc.dram_tensor("W", [2048, 256], F8, kind="ExternalInput").ap()
    a_b = nc.dram_tensor("b", [256], F32, kind="ExternalInput").ap()
    a_ones = nc.dram_tensor("ones", [128, 128], BF16, kind="ExternalInput").ap()
    a_rs = nc.dram_tensor("rowacc", [128, 48], F32, kind="ExternalOutput").ap()
    a_cs = nc.dram_tensor("colacc", [128, 3072], BF16, kind="ExternalOutput").ap()
    a_ps = nc.dram_tensor("possim", [1, 1024], F32, kind="ExternalOutput").ap()

    with tile.TileContext(nc) as tc:
        with tc.tile_pool(name="sb", bufs=1) as sb, \
             tc.tile_pool(name="emb", bufs=8) as embp, \
             tc.tile_pool(name="wk", bufs=2) as wk, \
             tc.tile_pool(name="cs", bufs=4) as csp, \
             tc.tile_pool(name="expp", bufs=3) as expp, \
             tc.tile_pool(name="headp", bufs=2, space="PSUM") as headp, \
             tc.tile_pool(name="simp", bufs=3, space="PSUM") as simp:

            t_ones = sb.tile([128, 128], BF16, name="t_ones")
            nc.sync.dma_start(t_ones[:], a_ones[:])
            t_b = sb.tile([128, 2], F32, name="t_b")
            nc.sync.dma_start(t_b[:], a_b.rearrange("(dh p) -> p dh", p=128))
            # fp8 DoubleRow weight layout: [ki, kk2, ko, d] with contraction
            # feature = kk2*256 + ko*128 + ki (the AP rearrange packs it).
            t_W = sb.tile([128, 8, 2, 256], F8, name="t_W")
            # W chunk 0 first so the first head matmul group can start as soon
            # as slab0/chunk0 lands; remaining W chunks queue behind it.
            nc.sync.dma_start(
                t_W[:, 0:2, :, :],
                a_W[0:512, :].rearrange("(kk2 ko p) d -> p kk2 ko d", ko=2, p=128))

            # HAM warm-up: dummy matmuls on the ones tile while the first slab
            # DMAs land, so the PE clock gate opens (1.2->2.4 GHz) before the
            # real head matmuls begin and stays open through the choppy start.
            p_wu = headp.tile([128, 512], F32, name="p_wu", tag="head")
            for _ in range(80):
                nc.tensor.matmul(p_wu[:, 0:128], t_ones[:], t_ones[:],
                                 start=True, stop=True)

            rp_st = sb.tile([128, 6, 8], F32, name="rp_st")
            ps_st = sb.tile([1, 1024], F32, name="ps_st")
            t_ln8 = sb.tile([128, 1], F32, name="t_ln8")
            nc.vector.memset(t_ln8[:], math.log(8.0))
            t_on = [sb.tile([128, 2, 1024], F8, name=f"t_on{k}") for k in range(4)]

            emb_tiles = {}

            def load_chunk(k, g):
                t_e = embp.tile([128, 2, 2, 1024], F8, name="t_e", tag="emb")
                src = a_emb[512 * g:512 * (g + 1), 1024 * k:1024 * (k + 1)]
                nc.sync.dma_start(
                    t_e[:], src.rearrange("(a ko p) r -> p a ko r", ko=2, p=128))
                emb_tiles.setdefault(k, []).append(t_e)

            def load_slab(k):
                for g in range(4):
                    load_chunk(k, g)

            def head_half(k, nb, t_h):
                for dh in range(2):
                    p_h = headp.tile([128, 512], F32, name="p_h", tag="head")
                    for kk2 in range(8):
                        g, a = divmod(kk2, 2)
                        nc.tensor.matmul(
                            p_h[:],
                            t_W[:, kk2, :, 128 * dh:128 * (dh + 1)],
                            emb_tiles[k][g][:, a, :, 512 * nb:512 * (nb + 1)],
                            start=(kk2 == 0), stop=(kk2 == 7),
                            perf_mode=DR,
                        )
                    nc.vector.tensor_scalar_add(
                        t_h[:, dh, 512 * nb:512 * (nb + 1)], p_h[:], t_b[:, dh:dh + 1])

            def sq_of(t_h, sq=None, nb=None):
                t_sq = sq if sq is not None else wk.tile(
                    [128, 2, 1024], BF16, name="t_sq", tag="sq")
                s = slice(0, 1024) if nb is None else slice(512 * nb, 512 * (nb + 1))
                nc.vector.tensor_tensor(t_sq[:, :, s], t_h[:, :, s],
                                        t_h[:, :, s], ALU.mult)
                return t_sq

            def ns_mm(t_sq, ns=None, nbs=(0, 1)):
                p_ns = ns if ns is not None else simp.tile(
                    [128, 1024], F32, name="p_ns", tag="sim")
                for nb in nbs:
                    for dh in range(2):
                        nc.tensor.matmul(
                            p_ns[:, 512 * nb:512 * (nb + 1)], t_ones[:],
                            t_sq[:, dh, 512 * nb:512 * (nb + 1)],
                            start=(dh == 0), stop=(dh == 1))
                return p_ns

            def norm_act(k, p_src, t_h, lnr=None, nb=None):
                if lnr is None:
                    t_ln = wk.tile([128, 1024], F32, name="t_ln", tag="ln")
                    t_r = wk.tile([128, 1024], F32, name="t_r", tag="r")
                else:
                    t_ln, t_r = lnr
                s = slice(0, 1024) if nb is None else slice(512 * nb, 512 * (nb + 1))
                nc.scalar.activation(t_ln[:, s], p_src, AF.Ln)
                # r = 8/||h||: the x8 biases u-hat into fp8's normal range;
                # the sim exp compensates with scale = 10/64.
                nc.scalar.activation(t_r[:, s], t_ln[:, s], AF.Exp, scale=-0.5,
                                     bias=t_ln8[:])
                for dh in range(2):
                    nc.vector.tensor_tensor(t_on[k][:, dh, s], t_h[:, dh, s],
                                            t_r[:, s], ALU.mult)
                return t_ln, t_r

            def block_gen(bslot, a, bm, colsum=True, rowsum_dve=False,
                          inject=None):
                """Generator: emits one sim tile per next() so tiles can be
                woven between head matmul quarters."""
                t_cs = None
                if colsum and bslot > 0:
                    t_cs = csp.tile([128, 1024], BF16, name=f"t_cs{bslot}", tag="cs")
                for mb in range(8):
                    if mb == 4 and inject is not None:
                        inject()   # ns+norm of a later slab: enters the ACT
                        # queue here so it isn't stuck behind this block's exps
                    # block 4 swaps orientation halfway: every core computes
                    # (0,3) rows 0-511 and (3,0) rows 512-1023, so all 8
                    # cores' rowsums are useful and no colsum pass is needed.
                    aa, bb = (a, bm) if (colsum or mb < 4) else (bm, a)
                    p_sim = simp.tile([128, 1024], F32, name="p_sim", tag="sim")
                    for nb in range(2):
                        nc.tensor.matmul(
                            p_sim[:, 512 * nb:512 * (nb + 1)],
                            t_on[aa][:, :, 128 * mb:128 * (mb + 1)],
                            t_on[bb][:, :, 512 * nb:512 * (nb + 1)],
                            start=True, stop=True, perf_mode=DR)
                    t_exp = expp.tile([128, 1024], BF16, name="t_exp", tag="exp")
                    if rowsum_dve:
                        # DVE is idle alongside this block: rowsums via DVE
                        # reduce, skipping the ACT accumulator read
                        nc.scalar.activation(t_exp[:], p_sim[:], AF.Exp,
                                             scale=0.15625)
                        nc.vector.tensor_reduce(rp_st[:, bslot, mb:mb + 1],
                                                t_exp[:], mybir.AxisListType.X,
                                                ALU.add)
                    else:
                        nc.scalar.activation(t_exp[:], p_sim[:], AF.Exp,
                                             scale=0.15625,
                                             accum_out=rp_st[:, bslot, mb:mb + 1])
                    if colsum and bslot > 0:
                        if mb == 0:
                            nc.vector.tensor_copy(t_cs[:], t_exp[:])
                        else:
                            nc.vector.tensor_tensor(t_cs[:], t_cs[:], t_exp[:], ALU.add)
                    yield
                if colsum and bslot > 0:
                    nc.sync.dma_start(a_cs[:, 1024 * (bslot - 1):1024 * bslot], t_cs[:])
                while True:
                    yield

            def run_block(gen):
                for _ in range(9):   # 8 tiles + epilogue (colsum DMA)
                    next(gen)

            def head_quarter(k, nb, dh, t_h):
                p_h = headp.tile([128, 512], F32, name="p_h", tag="head")
                for kk2 in range(8):
                    g, a = divmod(kk2, 2)
                    nc.tensor.matmul(
                        p_h[:],
                        t_W[:, kk2, :, 128 * dh:128 * (dh + 1)],
                        emb_tiles[k][g][:, a, :, 512 * nb:512 * (nb + 1)],
                        start=(kk2 == 0), stop=(kk2 == 7),
                        perf_mode=DR,
                    )
                nc.vector.tensor_scalar_add(
                    t_h[:, dh, 512 * nb:512 * (nb + 1)], p_h[:], t_b[:, dh:dh + 1])

            def weave_head(k, t_h, gen=None):
                """Emit head of slab k; if gen given, 2 block tiles ride after
                each of the 4 head quarters."""
                for nb in range(2):
                    for dh in range(2):
                        head_quarter(k, nb, dh, t_h)
                        if gen is not None:
                            next(gen)
                            next(gen)
                if gen is not None:
                    next(gen)   # epilogue (colsum DMA)

            t_h = [None] * 4
            t_sq = [None] * 4
            p_ns = [None] * 4

            def new_th(k):
                t_h[k] = wk.tile([128, 2, 1024], BF16, name=f"t_h{k}", tag="th")

            # ---- emission order tuned to keep the PE stream dense and to
            # ---- front-load block work (only 2 blocks after the last head)
            load_chunk(0, 0)
            nc.sync.dma_start(
                t_W[:, 2:8, :, :],
                a_W[512:2048, :].rearrange("(kk2 ko p) d -> p kk2 ko d",
                                           ko=2, p=128))
            for g in range(1, 4):
                load_chunk(0, g)
            def diag_qtile(mb, nbc):
                # quadrant tile of the diag block: stationary rows 128mb..,
                # moving = half nbc of slab 0.  rowsum partial -> DVE reduce.
                p_q = simp.tile([128, 1024], F32, name="p_q", tag="sim")
                nc.tensor.matmul(p_q[:, 0:512],
                                 t_on[0][:, :, 128 * mb:128 * (mb + 1)],
                                 t_on[0][:, :, 512 * nbc:512 * (nbc + 1)],
                                 start=True, stop=True, perf_mode=DR)
                t_exp = expp.tile([128, 1024], BF16, name="t_exp", tag="exp")
                nc.scalar.activation(t_exp[:, 0:512], p_q[:, 0:512], AF.Exp,
                                     scale=0.15625)
                nc.vector.tensor_reduce(
                    rp_st[:, 5 * nbc, mb:mb + 1], t_exp[:, 0:512],
                    mybir.AxisListType.X, ALU.add)

            def diag_ftile(mb):
                p_sim = simp.tile([128, 1024], F32, name="p_sim", tag="sim")
                for nb in range(2):
                    nc.tensor.matmul(
                        p_sim[:, 512 * nb:512 * (nb + 1)],
                        t_on[0][:, :, 128 * mb:128 * (mb + 1)],
                        t_on[0][:, :, 512 * nb:512 * (nb + 1)],
                        start=True, stop=True, perf_mode=DR)
                t_exp = expp.tile([128, 1024], BF16, name="t_exp", tag="exp")
                nc.scalar.activation(t_exp[:], p_sim[:], AF.Exp, scale=0.15625)
                nc.vector.tensor_reduce(rp_st[:, 0, mb:mb + 1], t_exp[:],
                                        mybir.AxisListType.X, ALU.add)

            # --- slab 0: half-slab normalize so the diag block starts ASAP.
            # Its normsq tiles come from the head pool (the sim pool's 3-deep
            # rotation would recycle a long-lived ns tile under the diag
            # quadrant tiles and deadlock the in-order PE stream).
            load_slab(1)
            new_th(0)
            t_sq[0] = wk.tile([128, 2, 1024], BF16, name="t_sq0", tag="sq")
            head_quarter(0, 0, 0, t_h[0])
            head_quarter(0, 0, 1, t_h[0])
            sq_of(t_h[0], sq=t_sq[0], nb=0)
            head_quarter(0, 1, 0, t_h[0])
            p_nsA = headp.tile([128, 512], F32, name="p_nsA", tag="head")
            for dh in range(2):
                nc.tensor.matmul(p_nsA[:], t_ones[:], t_sq[0][:, dh, 0:512],
                                 start=(dh == 0), stop=(dh == 1))
            head_quarter(0, 1, 1, t_h[0])
            lnr0 = norm_act(0, p_nsA[:], t_h[0], nb=0)
            for mb in range(4):
                diag_qtile(mb, 0)          # first exps ~13us earlier
            sq_of(t_h[0], sq=t_sq[0], nb=1)
            p_nsB = headp.tile([128, 512], F32, name="p_nsB", tag="head")
            for dh in range(2):
                nc.tensor.matmul(p_nsB[:], t_ones[:], t_sq[0][:, dh, 512:1024],
                                 start=(dh == 0), stop=(dh == 1))
            load_slab(2)
            new_th(1)
            head_quarter(1, 0, 0, t_h[1])
            norm_act(0, p_nsB[:], t_h[0], lnr=lnr0, nb=1)
            head_quarter(1, 0, 1, t_h[1])
            diag_qtile(0, 1); diag_qtile(1, 1)
            head_quarter(1, 1, 0, t_h[1])
            diag_qtile(2, 1); diag_qtile(3, 1)
            head_quarter(1, 1, 1, t_h[1])
            diag_ftile(4); diag_ftile(5)
            t_sq[1] = sq_of(t_h[1])
            p_ns[1] = ns_mm(t_sq[1])
            diag_ftile(6); diag_ftile(7)
            norm_act(1, p_ns[1][:], t_h[1])

            # --- slab 2 head with block (0,1) riding inside
            g1 = block_gen(1, *BLOCKS[1])
            load_slab(3)
            new_th(2)
            head_quarter(2, 0, 0, t_h[2])
            for _ in range(2): next(g1)
            head_quarter(2, 0, 1, t_h[2])
            for _ in range(2): next(g1)
            head_quarter(2, 1, 0, t_h[2])
            for _ in range(2): next(g1)
            head_quarter(2, 1, 1, t_h[2])
            t_sq[2] = sq_of(t_h[2])
            p_ns[2] = ns_mm(t_sq[2])
            for _ in range(3): next(g1)
            norm_act(2, p_ns[2][:], t_h[2])

            # --- slab 3 head with block (0,2) riding inside
            g2 = block_gen(2, *BLOCKS[2])
            new_th(3)
            head_quarter(3, 0, 0, t_h[3])
            for _ in range(2): next(g2)
            head_quarter(3, 0, 1, t_h[3])
            for _ in range(2): next(g2)
            head_quarter(3, 1, 0, t_h[3])
            for _ in range(2): next(g2)
            head_quarter(3, 1, 1, t_h[3])
            t_sq[3] = sq_of(t_h[3])
            p_ns[3] = ns_mm(t_sq[3])
            for _ in range(3): next(g2)
            norm_act(3, p_ns[3][:], t_h[3])

            # pos: elementwise product slabs slot0 x slot3, column sums over d
            # (emitted here so it overlaps blocks 3-4 instead of tailing)
            t_pp = wk.tile([128, 2, 1024], BF16, name="t_pp", tag="pp", bufs=1)
            nc.vector.tensor_tensor(t_pp[:], t_on[0][:], t_on[3][:], ALU.mult)
            for nb in range(2):
                p_ps = headp.tile([1, 512], F32, name=f"p_ps{nb}", tag="head")
                for dh in range(2):
                    nc.tensor.matmul(p_ps[:], t_ones[:, 0:1],
                                     t_pp[:, dh, 512 * nb:512 * (nb + 1)],
                                     start=(dh == 0), stop=(dh == 1))
                nc.vector.tensor_copy(ps_st[0:1, 512 * nb:512 * (nb + 1)], p_ps[:])
            nc.sync.dma_start(a_ps[:], ps_st[:])

            run_block(block_gen(3, *BLOCKS[3]))
            run_block(block_gen(4, *BLOCKS[4], colsum=False))

            # final DMA
            nc.sync.dma_start(a_rs[:], rp_st[:].rearrange("p a m -> p (a m)"))

    nc.compile()
    _CACHE["nc"] = nc
    return nc


def _host_inputs(embedded_data, W, b):
    import ml_dtypes
    f8 = ml_dtypes.float8_e4m3
    # W is scaled by 64 (dyadic) to land in fp8's normal range; the L2
    # normalize absorbs any per-row scale on the head output exactly.
    embT = np.asarray(embedded_data, dtype=np.float32).T.astype(f8)
    Wb = (np.asarray(W, dtype=np.float32) * 64.0).astype(f8)
    b64 = np.asarray(b, dtype=np.float32) * 64.0
    ones = np.ones((128, 128), dtype=ml_dtypes.bfloat16)
    in_maps = []
    for c in range(8):
        cols = np.ascontiguousarray(np.concatenate(
            [embT[:, 1024 * s:1024 * (s + 1)] for s in SLOTS[c]], axis=1))
        in_maps.append({"embT": cols, "W": Wb, "b": b64, "ones": ones})
    return in_maps


def _combine(results):
    neg = np.zeros(8192, np.float64)
    pos = np.zeros(8192, np.float64)
    E10 = np.exp(10.0)
    for c in range(8):
        S = SLOTS[c]
        rs = results[c]["rowacc"].astype(np.float64).reshape(128, 6, 8)
        cs = results[c]["colacc"].astype(np.float64).reshape(128, 3, 1024).sum(axis=0)
        rows = [rs[:, bl, :].T.reshape(-1) for bl in range(5)]
        # diag rows 0-511 were computed as two half-tile partials
        rows[0][:512] += rs[:, 5, 0:4].T.reshape(-1)
        sl = [np.s_[1024 * s:1024 * (s + 1)] for s in S]
        neg[sl[0]] += rows[0] - E10          # diag block, self-sim removed
        neg[sl[0]] += rows[1]; neg[sl[1]] += cs[0]   # B1 (0,1)
        neg[sl[0]] += rows[2]; neg[sl[2]] += cs[1]   # B2 (0,2)
        neg[sl[1]] += rows[3]; neg[sl[3]] += cs[2]   # B3 (1,3)
        # B4: (0,3) rows 0-511 on this core + (3,0) rows 512-1023;
        # all 8 cores contribute, no colsums needed.
        neg[1024 * S[0]:1024 * S[0] + 512] += rows[4][:512]
        neg[1024 * S[3] + 512:1024 * (S[3] + 1)] += rows[4][512:]
        # possim was computed on 8x-scaled u-hat vectors -> /64
        ps = results[c]["possim"].astype(np.float64).ravel() / 64.0
        pos[sl[0]] = ps
    loss = -np.mean(10.0 * pos - np.log(neg))
    return np.float32(loss)


def run(embedded_data, W, b, trace=False):
    from concourse import bass_utils
    nc = _build()
    in_maps = _host_inputs(embedded_data, W, b)
    res = bass_utils.run_bass_kernel_spmd(nc, in_maps, core_ids=list(range(8)),
                                          trace=trace)
    return _combine(res.results), res


def kernel(embedded_data, W, b):
    loss, _ = run(embedded_data, W, b, trace=False)
    return np.asarray(loss, dtype=np.float32)
